# revision 4
# baseline (speedup 1.0000x reference)
"""Trainium2 Bass kernel for nn_Linear_65747359367815.

Computation (see problem reference):
  x [B=256, N=4096, C=16, 2(r/i)]
  stage 1: per-channel length-L (=4087) sliding dot products at W=10 lags
           for real and imag parts  -> fr, fi [B, W, C]
  stage 2: amp = fr^2+fi^2 scaling, per-channel Linear(2C->2)
  stage 3: per-channel output filters over the W dim -> [B, C, 2]
  stage 4: BatchNorm1d over (batch, last) per channel

Strategy: pure data parallel over B across 8 NeuronCores (32 batch rows
per core). Stage 1 is ~99.99% of the FLOPs and all of the memory traffic
(128 MiB); it runs on device as TensorE matmuls contracting over n in 32
chunks of 128 partitions:

    out[j, b] += A_c[n_chunk, j].T @ X_c[n_chunk, b]

where A_c is a host-prebuilt Toeplitz block (A_c[n, j] = w_c[n - j]) and
X_c is the per-channel data, host-pretransposed into an [n, b]-major HBM
layout so every DMA descriptor is a 512 B contiguous row. The four
(channel-sub, real/imag) streams of a channel pair run as concurrent
matmuls on disjoint 32-column strips of the PE array (tile_position via
PSUM base-partition placement). Stages 2-4 are a few MFLOP on [256,10,16]
tensors and run on host after the gather.
"""

import sys
import types

import numpy as np

import concourse.bass as bass
import concourse.mybir as mybir
from concourse.tile import TileContext, ScopedClock
from concourse.bass_utils import run_bass_kernel_spmd

B, N, C, W = 256, 4096, 16, 10
L = N - W + 1  # 4087
EPS = 1e-5
NCORES = 8
BLOC = B // NCORES  # 32
NCHUNK = N // 128  # 32
CP = C // 2  # 8 channel pairs
FP32 = mybir.dt.float32


def _split_sync_waits(nc):
    """This container's walrus caps every instruction at ONE sync wait.
    Hoist extra waits onto preceding same-engine nops."""
    cur = nc.cur_bb.bb
    for fn in nc.m.functions:
        for bb in fn.blocks:
            insts = list(bb.instructions)
            if not any(
                i.sync_info and i.sync_info.on_wait and len(i.sync_info.on_wait) > 1
                for i in insts
            ):
                continue
            pre_cur = list(cur.instructions)
            newlist = []
            for inst in insts:
                si = inst.sync_info
                waits = list(si.on_wait) if si and si.on_wait else []
                if len(waits) > 1:
                    for w in waits[:-1]:
                        h = nc.engines[inst.engine].nop(nofuse=True)
                        nop_inst = h.ins
                        nsi = nop_inst.sync_info
                        if nsi is None:
                            nop_inst.sync_info = mybir.SyncInfo(
                                on_wait=[w], on_update=[]
                            )
                        else:
                            nsi.on_wait = [w]
                        newlist.append(nop_inst)
                    si.on_wait = [waits[-1]]
                newlist.append(inst)
            cur.instructions = pre_cur
            bb.instructions = newlist


def _patched_drain_and_barrier(self, tick_clock, wait_clock):
    drain_inst = self.nc.sync.drain()
    wait_clock.add_sem_waits(drain_inst.ins, ScopedClock({None: tick_clock.global_clock}))
    self.nc.all_engine_barrier()
    assert self.sems is not None
    popped = self.nc._tile_sem_poison_stack.pop()
    assert popped is self._sem_poison
    self.nc.clear_and_free_semaphores(list(self.sems.allocated().values()))
    self.nc.all_engine_barrier()
    _split_sync_waits(self.nc)


if getattr(TileContext, "_drain_patch_installed", False) is False:
    TileContext._drain_and_barrier = _patched_drain_and_barrier
    TileContext._drain_patch_installed = True


def _build_nc():
    nc = bass.Bass()
    # xt[cp, n, f] where f = (c_sub, ri, b) -> 128 floats = 512 B rows
    xt = nc.dram_tensor("xt", [CP, N, 128], FP32, kind="ExternalInput")
    # toep[cri, p, k*W + j] with cri = 2*c + ri; lhsT for chunk k is [:, kW:(k+1)W]
    toep = nc.dram_tensor("toep", [2 * C, 128, NCHUNK * W], FP32, kind="ExternalInput")
    # out[p, cp*32 + b] with p = 32*t + j, t = 2*c_sub + ri (rows j>=10 garbage)
    out = nc.dram_tensor("out", [128, CP * BLOC], FP32, kind="ExternalOutput")

    with TileContext(nc) as tc:
        with tc.tile_pool(name="wpool", bufs=1) as wpool, \
             tc.tile_pool(name="xpool", bufs=3) as xpool, \
             tc.tile_pool(name="opool", bufs=1) as opool, \
             tc.tile_pool(name="ppool", bufs=4, space="PSUM") as ppool:
            wt = []
            for cri in range(2 * C):
                w = wpool.tile([128, NCHUNK * W], FP32, tag=f"w{cri}")
                nc.sync.dma_start(out=w[:], in_=toep[cri])
                wt.append(w)
            ot = opool.tile([128, CP * BLOC], FP32)
            for cp in range(CP):
                xtile = xpool.tile([128, N], FP32)
                nc.sync.dma_start(
                    out=xtile[:].rearrange("p (k f) -> p k f", k=NCHUNK),
                    in_=xt[cp].rearrange("(k p) f -> p k f", p=128),
                )
                ps = ppool.tile([128, BLOC], FP32)
                for k in range(NCHUNK):
                    for t in range(4):
                        nc.tensor.matmul(
                            ps[32 * t : 32 * t + W, :],
                            wt[4 * cp + t][:, k * W : (k + 1) * W],
                            xtile[:, k * 128 + 32 * t : k * 128 + 32 * t + 32],
                            start=(k == 0),
                            stop=(k == NCHUNK - 1),
                            tile_position=(0, 32 * t),
                        )
                nc.vector.tensor_copy(
                    out=ot[:, cp * BLOC : (cp + 1) * BLOC], in_=ps[:]
                )
            nc.sync.dma_start(out=out[:], in_=ot[:])
    return nc


_NC_CACHE = {}
LAST_RESULTS = None


def _get_nc():
    if "nc" not in _NC_CACHE:
        _NC_CACHE["nc"] = _build_nc()
    return _NC_CACHE["nc"]


def kernel(x, W_in_real, W_in_imag, W_nl, W_out_real, W_out_imag, gamma, beta):
    global LAST_RESULTS
    x = np.asarray(x, dtype=np.float32)
    W_in_real = np.asarray(W_in_real, dtype=np.float32)
    W_in_imag = np.asarray(W_in_imag, dtype=np.float32)
    W_nl = np.asarray(W_nl, dtype=np.float32)
    W_out_real = np.asarray(W_out_real, dtype=np.float32)
    W_out_imag = np.asarray(W_out_imag, dtype=np.float32)
    gamma = np.asarray(gamma, dtype=np.float32)
    beta = np.asarray(beta, dtype=np.float32)

    # --- device input prep -------------------------------------------------
    # [m, b, n, cp, cs, ri] -> [m, cp, n, cs, ri, b]
    xs = x.reshape(NCORES, BLOC, N, CP, 2, 2)
    xt_all = np.ascontiguousarray(xs.transpose(0, 3, 2, 4, 5, 1)).reshape(
        NCORES, CP, N, 128
    )

    # Toeplitz blocks: big[c, ri, n, j] = w_{ri}[c, n - j] (0 <= n-j < L)
    big = np.zeros((C, 2, N, W), np.float32)
    for j in range(W):
        big[:, 0, j : j + L, j] = W_in_real
        big[:, 1, j : j + L, j] = W_in_imag
    toep = np.ascontiguousarray(
        big.reshape(C, 2, NCHUNK, 128, W).transpose(0, 1, 3, 2, 4)
    ).reshape(2 * C, 128, NCHUNK * W)

    nc = _get_nc()
    in_maps = [{"xt": xt_all[m], "toep": toep} for m in range(NCORES)]
    res = run_bass_kernel_spmd(nc, in_maps, list(range(NCORES)))
    LAST_RESULTS = res

    # --- gather stage-1 results -------------------------------------------
    fr = np.empty((B, W, C), np.float32)
    fi = np.empty((B, W, C), np.float32)
    for m in range(NCORES):
        O = res.results[m]["out"].reshape(4, 32, CP, BLOC)  # [t, j, cp, b]
        # t = 2*cs + ri ; c = 2*cp + cs
        fr[m * BLOC : (m + 1) * BLOC] = (
            O[0::2, :W].transpose(3, 1, 2, 0).reshape(BLOC, W, C)
        )
        fi[m * BLOC : (m + 1) * BLOC] = (
            O[1::2, :W].transpose(3, 1, 2, 0).reshape(BLOC, W, C)
        )

    # --- host epilogue (a few MFLOP) ---------------------------------------
    amp = fr * fr + fi * fi
    fr = amp * fr
    fi = amp * fi
    tf = np.concatenate([fr, fi], axis=-1)  # [B, W, 2C]
    nl = np.einsum("bwi,coi->bwco", tf, W_nl)  # [B, W, C, 2]
    nr, ni = nl[..., 0], nl[..., 1]
    out_r = np.einsum("bwc,cw->bc", nr, W_out_real)
    out_i = np.einsum("bwc,cw->bc", ni, W_out_imag)
    out = np.stack([out_r, out_i], axis=-1)  # [B, C, 2]
    mean = out.mean(axis=(0, 2), keepdims=True)
    var = out.var(axis=(0, 2), keepdims=True)
    result = (out - mean) / np.sqrt(var + EPS) * gamma[None, :, None] + beta[
        None, :, None
    ]
    return result.astype(np.float32)


# revision 8
# speedup vs baseline: 1.1130x; 1.1130x over previous
"""Trainium2 Bass kernel for nn_Linear_65747359367815.

Computation (see problem reference):
  x [B=256, N=4096, C=16, 2(r/i)]
  stage 1: per-channel length-L (=4087) sliding dot products at W=10 lags
           for real and imag parts  -> fr, fi [B, W, C]
  stage 2: amp = fr^2+fi^2 scaling, per-channel Linear(2C->2)
  stage 3: per-channel output filters over the W dim -> [B, C, 2]
  stage 4: BatchNorm1d over (batch, last) per channel

Strategy: pure data parallel over B across 8 NeuronCores (32 batch rows
per core). Stage 1 is ~99.99% of the FLOPs and all of the memory traffic
(128 MiB); it runs on device as TensorE matmuls contracting over n in 32
chunks of 128 partitions:

    out[j, b] += A_c[n_chunk, j].T @ X_c[n_chunk, b]

where A_c is a host-prebuilt Toeplitz block (A_c[n, j] = w_c[n - j]) and
X_c is the per-channel data, host-pretransposed into an [n, b]-major HBM
layout so every DMA descriptor is a 512 B contiguous row. The four
(channel-sub, real/imag) streams of a channel pair run as concurrent
matmuls on disjoint 32-column strips of the PE array (tile_position via
PSUM base-partition placement). Stages 2-4 are a few MFLOP on [256,10,16]
tensors and run on host after the gather.
"""

import sys
import types

import numpy as np

import concourse.bass as bass
import concourse.mybir as mybir
from concourse.tile import TileContext, ScopedClock
from concourse.bass_utils import run_bass_kernel_spmd

B, N, C, W = 256, 4096, 16, 10
L = N - W + 1  # 4087
EPS = 1e-5
NCORES = 8
BLOC = B // NCORES  # 32
NCHUNK = N // 128  # 32
CP = C // 2  # 8 channel pairs
FP32 = mybir.dt.float32


def _split_sync_waits(nc):
    """This container's walrus caps every instruction at ONE sync wait.
    Hoist extra waits onto preceding same-engine nops."""
    cur = nc.cur_bb.bb
    for fn in nc.m.functions:
        for bb in fn.blocks:
            insts = list(bb.instructions)
            if not any(
                i.sync_info and i.sync_info.on_wait and len(i.sync_info.on_wait) > 1
                for i in insts
            ):
                continue
            pre_cur = list(cur.instructions)
            newlist = []
            for inst in insts:
                si = inst.sync_info
                waits = list(si.on_wait) if si and si.on_wait else []
                if len(waits) > 1:
                    for w in waits[:-1]:
                        h = nc.engines[inst.engine].nop(nofuse=True)
                        nop_inst = h.ins
                        nsi = nop_inst.sync_info
                        if nsi is None:
                            nop_inst.sync_info = mybir.SyncInfo(
                                on_wait=[w], on_update=[]
                            )
                        else:
                            nsi.on_wait = [w]
                        newlist.append(nop_inst)
                    si.on_wait = [waits[-1]]
                newlist.append(inst)
            cur.instructions = pre_cur
            bb.instructions = newlist


def _patched_drain_and_barrier(self, tick_clock, wait_clock):
    drain_inst = self.nc.sync.drain()
    wait_clock.add_sem_waits(drain_inst.ins, ScopedClock({None: tick_clock.global_clock}))
    self.nc.all_engine_barrier()
    assert self.sems is not None
    popped = self.nc._tile_sem_poison_stack.pop()
    assert popped is self._sem_poison
    self.nc.clear_and_free_semaphores(list(self.sems.allocated().values()))
    self.nc.all_engine_barrier()
    _split_sync_waits(self.nc)


if getattr(TileContext, "_drain_patch_installed", False) is False:
    TileContext._drain_and_barrier = _patched_drain_and_barrier
    TileContext._drain_patch_installed = True


NSLAB = 16  # n-slabs of 2 chunks (256 n values); DMA rows are 4 KiB


def _build_nc():
    nc = bass.Bass()
    # xt[slab, n_in_slab, f] where f = (c, ri, b) -> 1024 floats = 4 KiB rows
    xt = nc.dram_tensor("xt", [NSLAB, 256, 1024], FP32, kind="ExternalInput")
    # toep[cri, p, k*W + j] with cri = 2*c + ri; lhsT for chunk k is [:, kW:(k+1)W]
    toep = nc.dram_tensor("toep", [2 * C, 128, NCHUNK * W], FP32, kind="ExternalInput")
    # out[p, q*32 + b] with p = 32*t + j, cri = 4*q + t (rows j>=10 garbage)
    out = nc.dram_tensor("out", [128, CP * BLOC], FP32, kind="ExternalOutput")

    with TileContext(nc) as tc:
        with tc.tile_pool(name="wpool", bufs=1) as wpool, \
             tc.tile_pool(name="xpool", bufs=1) as xpool, \
             tc.tile_pool(name="opool", bufs=1) as opool, \
             tc.tile_pool(name="ppool", bufs=1, space="PSUM") as ppool:
            wt = []
            for cri in range(2 * C):
                w = wpool.tile([128, NCHUNK * W], FP32, tag=f"w{cri}", name=f"w{cri}")
                nc.sync.dma_start(out=w[:], in_=toep[cri])
                wt.append(w)
            slabs = []
            for s in range(NSLAB):
                xtile = xpool.tile([128, 2048], FP32, tag=f"x{s}", name=f"x{s}")
                nc.sync.dma_start(
                    out=xtile[:].rearrange("p (k f) -> p k f", k=2),
                    in_=xt[s].rearrange("(k p) f -> p k f", p=128),
                )
                slabs.append(xtile)
            ot = opool.tile([128, CP * BLOC], FP32)
            pss = [ppool.tile([128, BLOC], FP32, tag=f"ps{q}", name=f"ps{q}") for q in range(8)]
            for k in range(NCHUNK):
                base = (k % 2) * 1024
                for cri in range(2 * C):
                    q, t = divmod(cri, 4)
                    nc.tensor.matmul(
                        pss[q][32 * t : 32 * t + W, :],
                        wt[cri][:, k * W : (k + 1) * W],
                        slabs[k // 2][:, base + cri * 32 : base + cri * 32 + 32],
                        start=(k == 0),
                        stop=(k == NCHUNK - 1),
                        tile_position=(0, 32 * t),
                    )
            for q in range(8):
                nc.vector.tensor_copy(
                    out=ot[:, q * BLOC : (q + 1) * BLOC], in_=pss[q][:]
                )
            nc.sync.dma_start(out=out[:], in_=ot[:])
    return nc


_NC_CACHE = {}
LAST_RESULTS = None


def _get_nc():
    if "nc" not in _NC_CACHE:
        _NC_CACHE["nc"] = _build_nc()
    return _NC_CACHE["nc"]


def kernel(x, W_in_real, W_in_imag, W_nl, W_out_real, W_out_imag, gamma, beta):
    global LAST_RESULTS
    x = np.asarray(x, dtype=np.float32)
    W_in_real = np.asarray(W_in_real, dtype=np.float32)
    W_in_imag = np.asarray(W_in_imag, dtype=np.float32)
    W_nl = np.asarray(W_nl, dtype=np.float32)
    W_out_real = np.asarray(W_out_real, dtype=np.float32)
    W_out_imag = np.asarray(W_out_imag, dtype=np.float32)
    gamma = np.asarray(gamma, dtype=np.float32)
    beta = np.asarray(beta, dtype=np.float32)

    # --- device input prep -------------------------------------------------
    # [m, b, n, c, ri] -> [m, n, c, ri, b] -> [m, slab, n_in_slab, 1024]
    xs = x.reshape(NCORES, BLOC, N, C, 2)
    xt_all = np.ascontiguousarray(xs.transpose(0, 2, 3, 4, 1)).reshape(
        NCORES, NSLAB, 256, 1024
    )

    # Toeplitz blocks: big[c, ri, n, j] = w_{ri}[c, n - j] (0 <= n-j < L)
    big = np.zeros((C, 2, N, W), np.float32)
    for j in range(W):
        big[:, 0, j : j + L, j] = W_in_real
        big[:, 1, j : j + L, j] = W_in_imag
    toep = np.ascontiguousarray(
        big.reshape(C, 2, NCHUNK, 128, W).transpose(0, 1, 3, 2, 4)
    ).reshape(2 * C, 128, NCHUNK * W)

    nc = _get_nc()
    in_maps = [{"xt": xt_all[m], "toep": toep} for m in range(NCORES)]
    res = run_bass_kernel_spmd(nc, in_maps, list(range(NCORES)))
    LAST_RESULTS = res

    # --- gather stage-1 results -------------------------------------------
    fr = np.empty((B, W, C), np.float32)
    fi = np.empty((B, W, C), np.float32)
    for m in range(NCORES):
        O = res.results[m]["out"].reshape(4, 32, CP, BLOC)  # [t, j, cp, b]
        # t = 2*cs + ri ; c = 2*cp + cs
        fr[m * BLOC : (m + 1) * BLOC] = (
            O[0::2, :W].transpose(3, 1, 2, 0).reshape(BLOC, W, C)
        )
        fi[m * BLOC : (m + 1) * BLOC] = (
            O[1::2, :W].transpose(3, 1, 2, 0).reshape(BLOC, W, C)
        )

    # --- host epilogue (a few MFLOP) ---------------------------------------
    amp = fr * fr + fi * fi
    fr = amp * fr
    fi = amp * fi
    tf = np.concatenate([fr, fi], axis=-1)  # [B, W, 2C]
    nl = np.einsum("bwi,coi->bwco", tf, W_nl)  # [B, W, C, 2]
    nr, ni = nl[..., 0], nl[..., 1]
    out_r = np.einsum("bwc,cw->bc", nr, W_out_real)
    out_i = np.einsum("bwc,cw->bc", ni, W_out_imag)
    out = np.stack([out_r, out_i], axis=-1)  # [B, C, 2]
    mean = out.mean(axis=(0, 2), keepdims=True)
    var = out.var(axis=(0, 2), keepdims=True)
    result = (out - mean) / np.sqrt(var + EPS) * gamma[None, :, None] + beta[
        None, :, None
    ]
    return result.astype(np.float32)


# revision 9
# speedup vs baseline: 1.7561x; 1.5778x over previous
"""Trainium2 Bass kernel for nn_Linear_65747359367815.

Computation (see problem reference):
  x [B=256, N=4096, C=16, 2(r/i)]
  stage 1: per-channel length-L (=4087) sliding dot products at W=10 lags
           for real and imag parts  -> fr, fi [B, W, C]
  stage 2: amp = fr^2+fi^2 scaling, per-channel Linear(2C->2)
  stage 3: per-channel output filters over the W dim -> [B, C, 2]
  stage 4: BatchNorm1d over (batch, last) per channel

Strategy: hybrid data parallel over the 8 NeuronCores — 4-way over batch
(64 rows/core) x 2-way over the contraction dim n (2048/core), partials
summed on host. Stage 1 is ~99.99% of the FLOPs and all of the memory
traffic (128 MiB); it runs on device as TensorE matmuls contracting over
n in chunks of 128 partitions:

    out[j, b] += A_c[n_chunk, j].T @ X_c[n_chunk, b]

where A_c is a host-prebuilt Toeplitz block (A_c[n, j] = w_c[n - j]) and
X is host-pretransposed into an [n, (c, ri, b)]-major HBM layout so every
DMA descriptor is an 8 KiB contiguous row. The 32 (channel, r/i) streams
accumulate into all 8 PSUM banks, 4 concurrent col-group strips per bank
(tile_position). Stages 2-4 are a few MFLOP on [256,10,16] tensors and
run on host after the gather.
"""

import numpy as np

import concourse.bass as bass
import concourse.mybir as mybir
from concourse.tile import TileContext, ScopedClock
from concourse.bass_utils import run_bass_kernel_spmd

B, N, C, W = 256, 4096, 16, 10
L = N - W + 1  # 4087
EPS = 1e-5
NCORES = 8
NSPLIT = 2  # n-way split of the contraction dim
NBATCH = NCORES // NSPLIT  # batch-parallel groups
BLOC = B // NBATCH  # batch rows per core
NLOC = N // NSPLIT  # contraction elements per core
NCHUNK = NLOC // 128  # 128-partition chunks per core
NSLAB = NCHUNK // 2  # DMA slabs of 2 chunks
FREE = 2 * C * BLOC  # free elements per n value (c, ri, b)
FP32 = mybir.dt.float32


def _split_sync_waits(nc):
    """This container's walrus caps every instruction at ONE sync wait.
    Hoist extra waits onto preceding same-engine nops."""
    cur = nc.cur_bb.bb
    for fn in nc.m.functions:
        for bb in fn.blocks:
            insts = list(bb.instructions)
            if not any(
                i.sync_info and i.sync_info.on_wait and len(i.sync_info.on_wait) > 1
                for i in insts
            ):
                continue
            pre_cur = list(cur.instructions)
            newlist = []
            for inst in insts:
                si = inst.sync_info
                waits = list(si.on_wait) if si and si.on_wait else []
                if len(waits) > 1:
                    for w in waits[:-1]:
                        h = nc.engines[inst.engine].nop(nofuse=True)
                        nop_inst = h.ins
                        nsi = nop_inst.sync_info
                        if nsi is None:
                            nop_inst.sync_info = mybir.SyncInfo(
                                on_wait=[w], on_update=[]
                            )
                        else:
                            nsi.on_wait = [w]
                        newlist.append(nop_inst)
                    si.on_wait = [waits[-1]]
                newlist.append(inst)
            cur.instructions = pre_cur
            bb.instructions = newlist


def _patched_drain_and_barrier(self, tick_clock, wait_clock):
    drain_inst = self.nc.sync.drain()
    wait_clock.add_sem_waits(drain_inst.ins, ScopedClock({None: tick_clock.global_clock}))
    self.nc.all_engine_barrier()
    assert self.sems is not None
    popped = self.nc._tile_sem_poison_stack.pop()
    assert popped is self._sem_poison
    self.nc.clear_and_free_semaphores(list(self.sems.allocated().values()))
    self.nc.all_engine_barrier()
    _split_sync_waits(self.nc)


if getattr(TileContext, "_drain_patch_installed", False) is False:
    TileContext._drain_and_barrier = _patched_drain_and_barrier
    TileContext._drain_patch_installed = True


def _build_nc():
    nc = bass.Bass()
    # xt[slab, n_in_slab, f] with f = (c, ri, b) -> 2048 floats = 8 KiB rows
    xt = nc.dram_tensor("xt", [NSLAB, 256, FREE], FP32, kind="ExternalInput")
    # toep[g, p, (cri8, k, j)]: weight group g holds cri = 8g..8g+7;
    # lhsT for (cri, k) = toep_tile[g][:, (cri%8)*NCHUNK*W + k*W : +W]
    toep = nc.dram_tensor(
        "toep", [4, 128, 8 * NCHUNK * W], FP32, kind="ExternalInput"
    )
    # out[p, q*BLOC + b] with p = 32*t + j, cri = 4*q + t (rows j>=10 garbage)
    out = nc.dram_tensor("out", [128, 8 * BLOC], FP32, kind="ExternalOutput")

    with TileContext(nc) as tc:
        with tc.tile_pool(name="wpool", bufs=1) as wpool, \
             tc.tile_pool(name="xpool", bufs=1) as xpool, \
             tc.tile_pool(name="opool", bufs=1) as opool, \
             tc.tile_pool(name="ppool", bufs=1, space="PSUM") as ppool:
            wt = []
            for g in range(4):
                w = wpool.tile([128, 8 * NCHUNK * W], FP32, tag=f"w{g}", name=f"w{g}")
                nc.sync.dma_start(out=w[:], in_=toep[g])
                wt.append(w)
            slabs = []
            for s in range(NSLAB):
                xtile = xpool.tile([128, 2 * FREE], FP32, tag=f"x{s}", name=f"x{s}")
                nc.sync.dma_start(
                    out=xtile[:].rearrange("p (k f) -> p k f", k=2),
                    in_=xt[s].rearrange("(k p) f -> p k f", p=128),
                )
                slabs.append(xtile)
            ot = opool.tile([128, 8 * BLOC], FP32)
            pss = [
                ppool.tile([128, BLOC], FP32, tag=f"ps{q}", name=f"ps{q}")
                for q in range(8)
            ]
            for k in range(NCHUNK):
                base = (k % 2) * FREE
                for cri in range(2 * C):
                    q, t = divmod(cri, 4)
                    nc.tensor.matmul(
                        pss[q][32 * t : 32 * t + W, :],
                        wt[cri // 8][
                            :,
                            (cri % 8) * NCHUNK * W + k * W :
                            (cri % 8) * NCHUNK * W + (k + 1) * W,
                        ],
                        slabs[k // 2][
                            :, base + cri * BLOC : base + (cri + 1) * BLOC
                        ],
                        start=(k == 0),
                        stop=(k == NCHUNK - 1),
                        tile_position=(0, 32 * t),
                    )
            for q in range(8):
                nc.vector.tensor_copy(
                    out=ot[:, q * BLOC : (q + 1) * BLOC], in_=pss[q][:]
                )
            nc.sync.dma_start(out=out[:], in_=ot[:])
    return nc


_NC_CACHE = {}
LAST_RESULTS = None


def _get_nc():
    if "nc" not in _NC_CACHE:
        _NC_CACHE["nc"] = _build_nc()
    return _NC_CACHE["nc"]


def kernel(x, W_in_real, W_in_imag, W_nl, W_out_real, W_out_imag, gamma, beta):
    global LAST_RESULTS
    x = np.asarray(x, dtype=np.float32)
    W_in_real = np.asarray(W_in_real, dtype=np.float32)
    W_in_imag = np.asarray(W_in_imag, dtype=np.float32)
    W_nl = np.asarray(W_nl, dtype=np.float32)
    W_out_real = np.asarray(W_out_real, dtype=np.float32)
    W_out_imag = np.asarray(W_out_imag, dtype=np.float32)
    gamma = np.asarray(gamma, dtype=np.float32)
    beta = np.asarray(beta, dtype=np.float32)

    # --- device input prep -------------------------------------------------
    # core = h*NBATCH + bg handles batch rows [bg*BLOC, (bg+1)*BLOC) and
    # contraction range [h*NLOC, (h+1)*NLOC)
    # [bg, b, h, n', c, ri] -> [bg, h, n', c, ri, b] -> slabs
    xs = x.reshape(NBATCH, BLOC, NSPLIT, NLOC, C, 2)
    xt_all = np.ascontiguousarray(xs.transpose(0, 2, 3, 4, 5, 1)).reshape(
        NBATCH, NSPLIT, NSLAB, 256, FREE
    )

    # Toeplitz blocks: big[c, ri, n, j] = w_{ri}[c, n - j] (0 <= n-j < L)
    big = np.zeros((C, 2, N, W), np.float32)
    for j in range(W):
        big[:, 0, j : j + L, j] = W_in_real
        big[:, 1, j : j + L, j] = W_in_imag
    # per n-split half: [cri, p, (k, j)] -> groups of 8 cri
    toeps = []
    for h in range(NSPLIT):
        th = (
            big[:, :, h * NLOC : (h + 1) * NLOC, :]
            .reshape(C, 2, NCHUNK, 128, W)
            .transpose(0, 1, 3, 2, 4)  # [c, ri, p, k, j]
            .reshape(2 * C, 128, NCHUNK * W)
            .transpose(1, 0, 2)  # [p, cri, (k j)]
            .reshape(128, 4, 8 * NCHUNK * W)
            .transpose(1, 0, 2)  # [g, p, (cri8 k j)]
        )
        toeps.append(np.ascontiguousarray(th))

    nc = _get_nc()
    in_maps = []
    for core in range(NCORES):
        h, bg = divmod(core, NBATCH)
        in_maps.append({"xt": xt_all[bg, h], "toep": toeps[h]})
    res = run_bass_kernel_spmd(nc, in_maps, list(range(NCORES)))
    LAST_RESULTS = res

    # --- gather stage-1 results (sum partials over n-splits) ---------------
    fr = np.zeros((B, W, C), np.float32)
    fi = np.zeros((B, W, C), np.float32)
    for core in range(NCORES):
        h, bg = divmod(core, NBATCH)
        O = res.results[core]["out"].reshape(4, 32, 8, BLOC)  # [t, j, q, b]
        # cri = 4q + t ; c = cri//2 = 2q + t//2 ; ri = t%2
        fr[bg * BLOC : (bg + 1) * BLOC] += (
            O[0::2, :W].transpose(3, 1, 2, 0).reshape(BLOC, W, C)
        )
        fi[bg * BLOC : (bg + 1) * BLOC] += (
            O[1::2, :W].transpose(3, 1, 2, 0).reshape(BLOC, W, C)
        )

    # --- host epilogue (a few MFLOP) ---------------------------------------
    amp = fr * fr + fi * fi
    fr = amp * fr
    fi = amp * fi
    tf = np.concatenate([fr, fi], axis=-1)  # [B, W, 2C]
    nl = np.einsum("bwi,coi->bwco", tf, W_nl)  # [B, W, C, 2]
    nr, ni = nl[..., 0], nl[..., 1]
    out_r = np.einsum("bwc,cw->bc", nr, W_out_real)
    out_i = np.einsum("bwc,cw->bc", ni, W_out_imag)
    out = np.stack([out_r, out_i], axis=-1)  # [B, C, 2]
    mean = out.mean(axis=(0, 2), keepdims=True)
    var = out.var(axis=(0, 2), keepdims=True)
    result = (out - mean) / np.sqrt(var + EPS) * gamma[None, :, None] + beta[
        None, :, None
    ]
    return result.astype(np.float32)
